# revision 54
# baseline (speedup 1.0000x reference)
"""CAML attention kernel for Trainium2 (8 NeuronCores, batch-sharded SPMD).

Reference computation:
    xt      = tanh(x)                      # [B, D, L]
    scores  = einsum('cd,bdl->bcl', W1, xt)
    weights = softmax(scores, axis=l)
    weighted= einsum('bcl,bdl->bcd', weights, xt)
    out     = einsum('cd,bcd->bc', W2, weighted) + b2

Key identity: the final contraction commutes with the softmax weighted sum,
so with s2 = einsum('cd,bdl->bcl', W2, xt):
    out[b,c] = (sum_l exp(s1[b,c,l]) * s2[b,c,l]) / (sum_l exp(s1[b,c,l])) + b2[c]
(|s1| <= 512*max|W1| ~ 13, so exp without max-subtraction is safe in fp32.)

Sharding: one batch element per core (x row-sliced), full C on every core.
C pads 8930 -> 8960 = 70 chunks of 128 (vs 9216 for a C-shard split), and
per-core HBM traffic drops from 41 MB (x replicated) to ~14 MB.

Both matmuls run fp8-e4m3 with DoubleRow (2 contraction rows per PE cell,
2x MAC throughput): per-core PE floor ~296 us vs ~600 us for fp16.
W1/W2 are scaled by 16 into e4m3's normal range; exp compensates with
scale=1/16 and the host divides the gathered output by 16.

L splits into 6 chunks (417*4 + 416*2) grouped in pairs; each pair's s1/s2
live in one 2-bank PSUM tile [128, 2, 512] so exp / the numerator product
run as one big ACT / DVE instruction per pair (amortizing the ~200-450
cycle per-instruction engine overhead) with accum_out producing the
softmax denominator / numerator partials directly.

Measured on HW (single-core trace, ~325us total): PE busy ~305us at
~98% main-loop occupancy (24 DoubleRow matmuls/j at the 417-cycle issue
floor), ACT ~253us, DVE ~244us hidden underneath; ~9us fixed NEFF init
+ ~7us teardown. Startup choreography: x bf16 (host cast) split across
the sync/scalar DMA queues, weights fp8 on the gpsimd queue (head chunk
for j<4 duplicated so j0's matmuls unblock at ~13us), blocks 2-4 gated
behind a j==4 data dependency to keep HBM free during the fill, and two
bursts of dummy matmuls (58 on a tiny early x-head chunk, 12 inside
j0's G2 bubble) keep the PE clock-gate (HAM) at 2.4GHz from ~12.5us
through the entire run.
"""

import numpy as np

import concourse.bacc as bacc
import concourse.tile as tile
from concourse import mybir
from concourse.bass import ts
from concourse.bass_utils import run_bass_kernel_spmd

B, D, L, C = 8, 512, 2500, 8930
N_CORES = 8
P = 128

C_PAD = 8960                 # 70 chunks of 128
JCH = C_PAD // P             # 70
KCH = D // P                 # 4 contraction chunks
NPAIR = KCH // 2             # 2 DoubleRow pairs
LTS = [417, 417, 417, 417, 416, 416]
LOFF = [0, 417, 834, 1251, 1668, 2084]
LCH = len(LTS)
NG = LCH // 2                # l-pairs per class chunk
JBLK = 14                    # j's per weight-DMA block (70 = 5*14)
NJB = JCH // JBLK            # 5 weight blocks
JHEAD = 4                    # j's in the duplicated head chunk
SLOT = 512                   # fp8 l-slot width (16B-aligned strides) & psum bank

F32 = mybir.dt.float32
BF16 = mybir.dt.bfloat16
BF16_NP = mybir.dt.np(mybir.dt.bfloat16)
FP8 = mybir.dt.float8e4
FP8_NP = mybir.dt.np(mybir.dt.float8e4)
W_SCALE = 16.0               # host scales W1/W2 (and b2) by this

FP8_S1 = False               # legacy knob for old test.py; ignored


def build_nc():
    """Emit the per-core program. All cores run the same NEFF (SPMD)."""
    nc = bacc.Bacc("TRN2", target_bir_lowering=False, debug=False)

    # DMA wants few, large, fully-contiguous transfers (<64KB is descriptor-
    # dominated): x in 3 per-l-pair tensors (~850KB, 6.7KB/partition lines),
    # weights in 5 per-j-block chunks (~920KB, 7.2KB/partition lines)
    xg = [
        nc.dram_tensor(f"x{g}", [P, KCH, LTS[2 * g] * 2], BF16,
                       kind="ExternalInput")
        for g in range(NG)
    ]
    w1t = nc.dram_tensor("w1t", [NJB, P, KCH, JBLK * P], FP8, kind="ExternalInput")
    w2t = nc.dram_tensor("w2t", [NJB, P, KCH, JBLK * P], FP8, kind="ExternalInput")
    # duplicated copy of the first JHEAD j-chunks so j0 can start ~10us
    # earlier than the full first block's arrival
    w1h = nc.dram_tensor("w1h", [P, KCH, JHEAD * P], FP8, kind="ExternalInput")
    w2h = nc.dram_tensor("w2h", [P, KCH, JHEAD * P], FP8, kind="ExternalInput")
    b2s = nc.dram_tensor("b2s", [P, JCH], F32, kind="ExternalInput")
    # tiny x head chunk: lands ~3us before the main x chunks so the PE
    # warm-up matmuls can start immediately after NEFF init
    xh = nc.dram_tensor("xh", [P, P], BF16, kind="ExternalInput")
    out = nc.dram_tensor("out", [P, JCH], F32, kind="ExternalOutput")

    Exp = mybir.ActivationFunctionType.Exp
    Tanh = mybir.ActivationFunctionType.Tanh
    mult = mybir.AluOpType.mult
    add = mybir.AluOpType.add
    DR = mybir.MatmulPerfMode.DoubleRow

    with tile.TileContext(nc) as tc:
        with (
            tc.tile_pool(name="wts", bufs=1) as wpool,
            tc.tile_pool(name="xraw", bufs=1) as xpool,
            tc.tile_pool(name="ps1", bufs=2, space="PSUM") as ppool1,
            tc.tile_pool(name="ps2", bufs=2, space="PSUM") as ppool2,
            tc.tile_pool(name="etile", bufs=4) as epool,
            tc.tile_pool(name="prod", bufs=3) as spool,
            tc.tile_pool(name="cols", bufs=2) as cpool,
            tc.tile_pool(name="outp", bufs=1) as opool,
        ):
            w1sb = wpool.tile([P, NJB, KCH, JBLK * P], FP8)
            w2sb = wpool.tile([P, NJB, KCH, JBLK * P], FP8)
            b2sb = wpool.tile([P, JCH], F32)
            # fp8 rhs for both matmuls: [part, k, l-slot, 512] -- all DoubleRow
            # middle-dim strides/offsets stay 16B-aligned via the 512 slots
            xt8 = wpool.tile([P, KCH, LCH, SLOT], FP8)
            out_all = opool.tile([P, JCH], F32)

            # three DMA queues in parallel: x split per l-pair across sync
            # (k 0-1) and scalar (k 2-3); weights + b2 on the gpsimd queue
            def dma_wblock(jb, eng):
                eng.dma_start(out=w1sb[:, jb], in_=w1t[jb])
                eng.dma_start(out=w2sb[:, jb], in_=w2t[jb])

            HP = JHEAD * P
            nc.gpsimd.dma_start(out=w1sb[:, 0, :, 0:HP], in_=w1h[:])
            nc.gpsimd.dma_start(out=w2sb[:, 0, :, 0:HP], in_=w2h[:])
            # rest of block 0 (strided source slice, lands later)
            nc.gpsimd.dma_start(
                out=w1sb[:, 0, :, HP:], in_=w1t[0, :, :, HP:])
            nc.gpsimd.dma_start(
                out=w2sb[:, 0, :, HP:], in_=w2t[0, :, :, HP:])
            nc.gpsimd.dma_start(out=b2sb, in_=b2s[:])

            xraws = {}
            xhsb = xpool.tile([P, P], BF16, tag="xh")
            nc.sync.dma_start(out=xhsb, in_=xh[:])

            def dma_xpair(g):
                xraw = xpool.tile([P, KCH, 2, LTS[2 * g]], BF16, tag=f"x{g}")
                nc.sync.dma_start(out=xraw[:, 0:2], in_=xg[g][:, 0:2])
                nc.scalar.dma_start(out=xraw[:, 2:4], in_=xg[g][:, 2:4])
                xraws[g] = xraw

            def tanh_pair(g):
                # two half-ops paired with the two x DMA queues: the k01
                # half unblocks the first DoubleRow pair's matmuls while
                # the k23 half is still converting
                lt = LTS[2 * g]
                for ks in (slice(0, 2), slice(2, 4)):
                    nc.scalar.activation(
                        out=xt8[:, ks, 2 * g : 2 * g + 2, 0:lt],
                        in_=xraws[g][:, ks], func=Tanh,
                    )

            dma_xpair(0)
            dma_xpair(1)
            dma_xpair(2)
            tanh_pair(0)
            tanh_pair(1)
            dma_wblock(1, nc.gpsimd)
            # blocks 2-4 are gated inside the j-loop (at j==5) so their
            # 5.5MB doesn't contend with x for HBM during the pipeline fill

            # HAM warm-up: ~3.5us of dummy matmuls gated on the x head chunk
            # (arrives right after NEFF init), so the clock-gate lifts the
            # 1.2GHz throttle before the first real matmul is ready
            warm = ppool1.tile([P, 2, SLOT], F32, tag="s1")
            for _ in range(58):
                nc.tensor.matmul(
                    warm[:, 0, 0:P],
                    xhsb,
                    xhsb,
                    start=True, stop=True,
                )

            prev_dcols = None
            for j in range(JCH):
                if j == 5:
                    # data-dependent gate: delays weight blocks 2-4 until
                    # the fill-critical x/weight transfers have finished
                    gate = cpool.tile([P, NG], F32, tag="gate")
                    nc.gpsimd.tensor_tensor(gate, prev_dcols, prev_dcols, add)
                    for jb in range(2, NJB):
                        dma_wblock(jb, nc.gpsimd)
                dcols = cpool.tile([P, NG], F32, tag="dcols")
                ncols = cpool.tile([P, NG], F32, tag="ncols")
                for g in range(NG):
                    la, lb = 2 * g, 2 * g + 1
                    lt = LTS[la]  # == LTS[lb]
                    s1g = ppool1.tile([P, 2, SLOT], F32, tag="s1")
                    s2g = ppool2.tile([P, 2, SLOT], F32)
                    if j == 0 and g == 2:
                        # keep the PE busy through the G2 bubble (waiting on
                        # tanh g2 behind two exps on ACT) so the HAM clock
                        # gate doesn't re-throttle mid-fill; the dummies
                        # write into this group's own s1 tile, which the
                        # real start=True matmuls below overwrite
                        for _ in range(12):
                            nc.tensor.matmul(
                                s1g[:, 0, 0:417],
                                xraws[0][:, 0, 0, 0:128],
                                xraws[0][:, 0, 0, 0:417],
                                start=True, stop=True,
                            )
                        # g2 tanh lands here: after j0's G0/G1 matmuls (so
                        # exp(j0,G0) isn't queued behind it on ACT) and
                        # before the G2 matmuls that consume it
                        tanh_pair(2)
                    # weight-major order: each stationary operand feeds the
                    # pair's two matmuls back-to-back (relieves LDWEIGHTS)
                    jb, jr = divmod(j, JBLK)
                    for wsb, sg in ((w1sb, s1g), (w2sb, s2g)):
                        for pr in range(NPAIR):
                            wsl = wsb[:, jb, 2 * pr : 2 * pr + 2, ts(jr, P)]
                            for li, l in ((0, la), (1, lb)):
                                nc.tensor.matmul(
                                    sg[:, li, 0:lt],
                                    wsl,
                                    xt8[:, 2 * pr : 2 * pr + 2, l, 0:lt],
                                    start=(pr == 0),
                                    stop=(pr == NPAIR - 1),
                                    perf_mode=DR,
                                )
                    e = epool.tile([P, 2, SLOT], F32, tag="e")
                    nc.scalar.activation(
                        out=e[:, :, 0:lt], in_=s1g[:, :, 0:lt], func=Exp,
                        scale=1.0 / W_SCALE,
                        accum_out=dcols[:, g : g + 1],
                    )
                    prod = spool.tile([P, 2, SLOT], F32, tag="prod")
                    nc.vector.scalar_tensor_tensor(
                        out=prod[:, :, 0:lt], in0=e[:, :, 0:lt], scalar=1.0,
                        in1=s2g[:, :, 0:lt], op0=mult, op1=mult,
                        accum_out=ncols[:, g : g + 1],
                    )
                denom = cpool.tile([P, 1], F32, tag="dsum")
                numer = cpool.tile([P, 1], F32, tag="nsum")
                recip = cpool.tile([P, 1], F32, tag="rsum")
                # tiny column reduces on DVE (cheap accumulator reads there;
                # on ACT each accum_out read costs ~280ns extra)
                dscr = cpool.tile([P, NG], F32, tag="dscr")
                nc.vector.tensor_scalar(dscr, dcols, 1.0, 0.0,
                                        mult, add, accum_out=denom)
                nscr = cpool.tile([P, NG], F32, tag="nscr")
                nc.vector.tensor_scalar(nscr, ncols, 1.0, 0.0,
                                        mult, add, accum_out=numer)
                nc.vector.reciprocal(recip, denom)
                # out = numer * (1/denom) + 16*b2   (everything 16x, host /16)
                nc.vector.scalar_tensor_tensor(
                    out=out_all[:, j : j + 1],
                    in0=numer, scalar=recip, in1=b2sb[:, j : j + 1],
                    op0=mult, op1=add,
                )
                prev_dcols = dcols
            nc.sync.dma_start(out=out[:], in_=out_all)

    nc.compile()
    return nc


_NC_CACHE = {}


def _get_nc():
    if "nc" not in _NC_CACHE:
        _NC_CACHE["nc"] = build_nc()
    return _NC_CACHE["nc"]


def make_in_maps(x, W1, W2, b2):
    """Host-side prep: pad C, pre-transpose + 16x-scale weights, cast fp8."""
    # x -> one [B, P, KCH, 2*lt] bf16 tensor per l-pair, fully contiguous
    xr = np.asarray(x, dtype=np.float32).reshape(B, KCH, P, L)
    xgs = []
    for g in range(NG):
        lo = LOFF[2 * g]
        span = 2 * LTS[2 * g]
        xgs.append(
            np.ascontiguousarray(
                xr[:, :, :, lo : lo + span].transpose(0, 2, 1, 3)
            ).astype(BF16_NP)
        )

    def prep_w(W):
        Wp = np.zeros((C_PAD, D), dtype=np.float32)
        Wp[:C] = np.asarray(W, dtype=np.float32) * W_SCALE
        Wt = np.ascontiguousarray(Wp.T).reshape(KCH, P, NJB, JBLK * P)
        # -> [NJB, P, KCH, JBLK*P], each jb block contiguous
        return np.ascontiguousarray(Wt.transpose(2, 1, 0, 3)).astype(FP8_NP)

    w1c, w2c = prep_w(W1), prep_w(W2)
    w1hc = np.ascontiguousarray(w1c[0, :, :, 0 : JHEAD * P])
    w2hc = np.ascontiguousarray(w2c[0, :, :, 0 : JHEAD * P])
    b2p = np.zeros((C_PAD,), dtype=np.float32)
    b2p[:C] = np.asarray(b2, dtype=np.float32) * W_SCALE
    b2c = np.ascontiguousarray(b2p.reshape(JCH, P).T)

    xh = np.ascontiguousarray(xgs[0][:, :, 0, 0:P])

    return [
        {
            "x0": xgs[0][i], "x1": xgs[1][i], "x2": xgs[2][i], "xh": xh[i],
            "w1t": w1c, "w2t": w2c, "w1h": w1hc, "w2h": w2hc, "b2s": b2c,
        }
        for i in range(N_CORES)
    ]


def gather_out(results):
    """results: list (per core) of {'out': [P, JCH]} -> full [B, C]."""
    rows = [
        np.asarray(r["out"], dtype=np.float32).T.reshape(C_PAD)[:C] / W_SCALE
        for r in results
    ]
    return np.stack(rows, axis=0)


def kernel(x, W1, W2, b2):
    nc = _get_nc()
    in_maps = make_in_maps(x, W1, W2, b2)
    res = run_bass_kernel_spmd(nc, in_maps, list(range(N_CORES)))
    return gather_out(res.results)
